# revision 10
# baseline (speedup 1.0000x reference)
"""CategoryDense (nn_CategoryDense) TRN2 Bass kernel.

out[b, c, o] = sum_i x[b, c, i] * kernel[0, c, i, o] + bias[0, c, o]
x: [8192, 64, 64] f32; kernel: [1, 64, 64, 64]; bias: [1, 64, 64].

Data-parallel over 8 NeuronCores: batch dim sharded 1024 rows/core,
weights + bias replicated; no cross-core communication.

Per-core kernel (Tile framework), per 128-row b-tile of x ([128, 4096]):
  - PE-transpose each [128 b, 128 (c,i)] column block (category pair
    2j, 2j+1) into PSUM; copy to SBUF as xT [128 (c,i), 128 b],
    rounding to float32r (single-pass PE dtype, ~fp22 multiply).
  - One matmul per pair against a block-diagonal [128, 128] float32r
    weight stack (cats 2j / 2j+1 on the two diagonal blocks):
      psum[b, 0:64]   = x[b, 2j]   @ W[2j]
      psum[b, 64:128] = x[b, 2j+1] @ W[2j+1]
  - DVE adds partition-broadcast bias while copying PSUM -> out tile.
  - Out tile [128, 4096] DMAs back contiguously.

float32r halves PE work vs fp32 (one pass instead of hi/lo two-pass);
inputs must be rounded to f32r by their producing instruction (the DVE
copy for xT, a casting gpsimd DMA for the weights).
"""

from contextlib import ExitStack

import numpy as np

import concourse.bass as bass  # noqa: F401  (engine namespaces live on nc)
import concourse.mybir as mybir
import concourse.tile as tile
from concourse import bacc
from concourse.bass_utils import run_bass_kernel_spmd
from concourse.masks import make_identity

F32 = mybir.dt.float32
F32R = mybir.dt.float32r

N_CORES = 8
B, C, IN, OUT = 8192, 64, 64, 64
B_SHARD = B // N_CORES


def _build_nc(b_shard=B_SHARD, xt_engines=("scalar", "vector")):
    n_btiles = b_shard // 128
    n_pairs = C // 2
    CI = C * IN
    CO = C * OUT

    nc = bacc.Bacc("TRN2", target_bir_lowering=False, debug=False)
    x = nc.dram_tensor("x", [b_shard, C, IN], F32, kind="ExternalInput").ap()
    # Host-prepared block-diagonal weight stacks (see kernel() below)
    wstack = nc.dram_tensor("wstack", [128, C // 2, 128], F32,
                            kind="ExternalInput").ap()
    bias = nc.dram_tensor("bias", [1, C, OUT], F32, kind="ExternalInput").ap()
    out = nc.dram_tensor("out", [b_shard, C, OUT], F32, kind="ExternalOutput").ap()

    x_t = x.rearrange("(t p) c i -> t p (c i)", p=128)
    out_t = out.rearrange("(t p) c o -> t p (c o)", p=128)

    with tile.TileContext(nc) as tc, ExitStack() as ctx:
        const_pool = ctx.enter_context(tc.tile_pool(name="const", bufs=1))
        x_pool = ctx.enter_context(tc.tile_pool(name="x", bufs=2))
        out_pool = ctx.enter_context(tc.tile_pool(name="out", bufs=2))
        xt_pool = ctx.enter_context(tc.tile_pool(name="xt", bufs=4))
        psum_t = ctx.enter_context(tc.tile_pool(name="psum_t", bufs=3, space="PSUM"))
        psum_o = ctx.enter_context(tc.tile_pool(name="psum_o", bufs=3, space="PSUM"))

        ident = const_pool.tile([128, 128], F32)
        make_identity(nc, ident)

        # Block-diagonal weight stacks; the casting gpsimd DMA rounds to f32r
        w_all = const_pool.tile([128, n_pairs, 128], F32R)
        nc.gpsimd.dma_start(w_all[:], wstack[:])

        # Bias replicated across all 128 partitions: [128, C*OUT]
        bias_sb = const_pool.tile([128, CO], F32)
        nc.sync.dma_start(
            bias_sb[:], bias.rearrange("a c o -> a (c o)").partition_broadcast(128)
        )

        for t in range(n_btiles):
            xt_sb = x_pool.tile([128, CI], F32)
            nc.sync.dma_start(xt_sb[:], x_t[t])
            o_sb = out_pool.tile([128, CO], F32)
            for j in range(n_pairs):
                ps_x = psum_t.tile([128, 128], F32)
                nc.tensor.transpose(ps_x[:], xt_sb[:, j * 128:(j + 1) * 128], ident[:])
                xT = xt_pool.tile([128, 128], F32R)
                if xt_engines[j % len(xt_engines)] == "scalar":
                    nc.scalar.copy(xT[:], ps_x[:])
                else:
                    nc.vector.tensor_copy(out=xT[:], in_=ps_x[:])
                ps_o = psum_o.tile([128, 128], F32)
                nc.tensor.matmul(ps_o[:], lhsT=xT[:], rhs=w_all[:, j],
                                 start=True, stop=True)
                nc.vector.tensor_add(out=o_sb[:, j * 128:(j + 1) * 128],
                                     in0=ps_o[:],
                                     in1=bias_sb[:, j * 128:(j + 1) * 128])
            nc.sync.dma_start(out_t[t], o_sb[:])

    nc.compile()
    return nc


_NC_CACHE = {}


def _get_nc():
    if "nc" not in _NC_CACHE:
        _NC_CACHE["nc"] = _build_nc()
    return _NC_CACHE["nc"]


def _install_ntff_shim():
    """Profiling only: register the axon NTFF hook under antenv.axon_hooks.

    The container's antenv stub lacks axon_hooks, so bass_utils'
    `from antenv.axon_hooks import get_axon_ntff_profile_hook` raises on
    trace=True runs. Recreate the module from trn_agent_boot's ctypes hook.
    """
    import sys
    import types

    if "antenv.axon_hooks" in sys.modules:
        return
    from trn_agent_boot.trn_boot import _ntff_profile_via_ctypes

    hook = _ntff_profile_via_ctypes("/opt/axon/libaxon_pjrt.so")
    mod = types.ModuleType("antenv.axon_hooks")
    mod.get_axon_ntff_profile_hook = lambda: hook
    mod.set_axon_ntff_profile_hook = lambda h: None
    sys.modules["antenv.axon_hooks"] = mod
    import antenv

    antenv.axon_hooks = mod


def kernel(x, kernel, bias, _trace=False, _trace_kwargs=None):
    x = np.ascontiguousarray(x, dtype=np.float32)
    kernel = np.ascontiguousarray(kernel, dtype=np.float32)
    bias = np.ascontiguousarray(bias, dtype=np.float32)
    assert x.shape == (B, C, IN)

    if _trace:
        _install_ntff_shim()
    nc = _get_nc()
    # Block-diagonal weight stacks: wstack[p, j, q] holds cat 2j at
    # [0:64, j, 0:64] and cat 2j+1 at [64:128, j, 64:128].
    wstack = np.zeros((128, C // 2, 128), dtype=np.float32)
    wstack[0:IN, :, 0:OUT] = kernel[0, 0::2].transpose(1, 0, 2)
    wstack[IN:128, :, OUT:128] = kernel[0, 1::2].transpose(1, 0, 2)
    in_maps = [
        {
            "x": x[i * B_SHARD:(i + 1) * B_SHARD],
            "wstack": wstack,
            "bias": bias,
        }
        for i in range(N_CORES)
    ]
    res = run_bass_kernel_spmd(
        nc, in_maps, core_ids=list(range(N_CORES)),
        trace=_trace, **(_trace_kwargs or {})
    )
    out = np.concatenate([res.results[i]["out"] for i in range(N_CORES)], axis=0)
    if _trace:
        _NC_CACHE["last_results"] = res
    return out


# revision 38
# speedup vs baseline: 1.5742x; 1.5742x over previous
"""CategoryDense (nn_CategoryDense) TRN2 Bass kernel.

out[b, c, o] = sum_i x[b, c, i] * kernel[0, c, i, o] + bias[0, c, o]
x: [8192, 64, 64] f32; kernel: [1, 64, 64, 64]; bias: [1, 64, 64].

Data-parallel over 8 NeuronCores: batch dim sharded 1024 rows/core,
weights + bias replicated; no cross-core communication.

Per-core kernel (Tile framework), per 128-row b-tile of x ([128, 4096]):
  - PE-transpose each [128 b, 128 (c,i)] column block (category pair
    2j, 2j+1) into PSUM; copy to SBUF as xT [128 (c,i), 128 b],
    rounding to float32r (single-pass PE dtype, ~fp22 multiply).
  - One matmul per pair against a block-diagonal [128, 128] float32r
    weight stack (cats 2j / 2j+1 on the two diagonal blocks):
      psum[b, 0:64]   = x[b, 2j]   @ W[2j]
      psum[b, 64:128] = x[b, 2j+1] @ W[2j+1]
  - DVE adds partition-broadcast bias while copying PSUM -> out tile.
  - Out tile [128, 4096] DMAs back contiguously.

float32r halves PE work vs fp32 (one pass instead of hi/lo two-pass);
inputs must be rounded to f32r by their producing instruction (the
PSUM->SBUF copy for xT, a one-time DVE cast for the weight stacks).
"""

from contextlib import ExitStack

import numpy as np

import concourse.bass as bass  # noqa: F401  (engine namespaces live on nc)
import concourse.mybir as mybir
import concourse.tile as tile
from concourse import bacc
from concourse.bass_utils import run_bass_kernel_spmd


F32 = mybir.dt.float32
F32R = mybir.dt.float32r

N_CORES = 8
B, C, IN, OUT = 8192, 64, 64, 64
B_SHARD = B // N_CORES


def _build_nc(b_shard=B_SHARD, xt_engines=("scalar", "scalar", "vector"),
              xt_bufs=32, psum_t_bufs=4, psum_o_bufs=4):
    n_btiles = b_shard // 128
    n_pairs = C // 2
    CI = C * IN
    CO = C * OUT

    nc = bacc.Bacc("TRN2", target_bir_lowering=False, debug=False)
    x = nc.dram_tensor("x", [b_shard, C, IN], F32, kind="ExternalInput").ap()
    # Host-prepared block-diagonal weight stacks (see kernel() below)
    wstack = nc.dram_tensor("wstack", [128, C // 2, 128], F32,
                            kind="ExternalInput").ap()
    bias = nc.dram_tensor("bias", [1, C, OUT], F32, kind="ExternalInput").ap()
    ident_in = nc.dram_tensor("ident", [128, 128], F32, kind="ExternalInput").ap()
    out = nc.dram_tensor("out", [b_shard, C, OUT], F32, kind="ExternalOutput").ap()

    x_t = x.rearrange("(t p) c i -> t p (c i)", p=128)
    out_t = out.rearrange("(t p) c o -> t p (c o)", p=128)

    with tile.TileContext(nc) as tc, ExitStack() as ctx:
        const_pool = ctx.enter_context(tc.tile_pool(name="const", bufs=1))
        x_pool = ctx.enter_context(tc.tile_pool(name="x", bufs=3))
        out_pool = ctx.enter_context(tc.tile_pool(name="out", bufs=3))
        xt_pool = ctx.enter_context(tc.tile_pool(name="xt", bufs=xt_bufs))
        psum_t = ctx.enter_context(
            tc.tile_pool(name="psum_t", bufs=psum_t_bufs, space="PSUM"))
        psum_o = ctx.enter_context(
            tc.tile_pool(name="psum_o", bufs=psum_o_bufs, space="PSUM"))

        # ident rides the SP ring ahead of the first x tile (tiny); the
        # big constants (wstack, bias) go on the ACT HWDGE ring so they
        # don't head-of-line block the x loads at startup.
        ident = const_pool.tile([128, 128], F32)
        nc.sync.dma_start(ident[:], ident_in[:])

        # Block-diagonal weight stacks; HWDGE load + one DVE cast rounds to
        # f32r (keeps the slow SWDGE path entirely off the pipeline start)
        w_stage = const_pool.tile([128, n_pairs, 128], F32)
        nc.scalar.dma_start(w_stage[:], wstack[:])
        w_all = const_pool.tile([128, n_pairs, 128], F32R)
        nc.vector.tensor_copy(out=w_all[:], in_=w_stage[:])

        # Bias replicated across all 128 partitions: [128, C*OUT]
        bias_sb = const_pool.tile([128, CO], F32)
        nc.scalar.dma_start(
            bias_sb[:], bias.rearrange("a c o -> a (c o)").partition_broadcast(128)
        )

        def emit_transpose(xt_sb, j):
            ps_x = psum_t.tile([128, 128], F32)
            nc.tensor.transpose(ps_x[:], xt_sb[:, j * 128:(j + 1) * 128], ident[:])
            xT = xt_pool.tile([128, 128], F32R)
            if xt_engines[j % len(xt_engines)] == "scalar":
                nc.scalar.copy(xT[:], ps_x[:])
            else:
                nc.vector.tensor_copy(out=xT[:], in_=ps_x[:])
            return xT

        def emit_matmul(o_sb, xT, j):
            ps_o = psum_o.tile([128, 128], F32)
            nc.tensor.matmul(ps_o[:], lhsT=xT[:], rhs=w_all[:, j],
                             start=True, stop=True)
            nc.vector.tensor_add(out=o_sb[:, j * 128:(j + 1) * 128],
                                 in0=ps_o[:],
                                 in1=bias_sb[:, j * 128:(j + 1) * 128])

        for t in range(n_btiles):
            xt_sb = x_pool.tile([128, CI], F32)
            if t == 0:
                # Quarter-split the first load so the transpose pipeline
                # starts ~3 quarters of a DMA earlier.
                q = CI // 4
                for k in range(4):
                    nc.sync.dma_start(xt_sb[:, k * q:(k + 1) * q],
                                      x_t[t][:, k * q:(k + 1) * q])
            else:
                nc.sync.dma_start(xt_sb[:], x_t[t])
            o_sb = out_pool.tile([128, CO], F32)
            xts = [emit_transpose(xt_sb, j) for j in range(n_pairs)]
            for j in range(n_pairs):
                emit_matmul(o_sb, xts[j], j)
            if t == n_btiles - 1:
                # Quarter-split the last store so it drains as the final
                # adds complete instead of waiting for the whole tile.
                q = CO // 4
                for k in range(4):
                    nc.sync.dma_start(out_t[t][:, k * q:(k + 1) * q],
                                      o_sb[:, k * q:(k + 1) * q])
            else:
                nc.sync.dma_start(out_t[t], o_sb[:])

    nc.compile()
    return nc


_NC_CACHE = {}


def _get_nc():
    if "nc" not in _NC_CACHE:
        _NC_CACHE["nc"] = _build_nc()
    return _NC_CACHE["nc"]


def _install_ntff_shim():
    """Profiling only: register the axon NTFF hook under antenv.axon_hooks.

    The container's antenv stub lacks axon_hooks, so bass_utils'
    `from antenv.axon_hooks import get_axon_ntff_profile_hook` raises on
    trace=True runs. Recreate the module from trn_agent_boot's ctypes hook.
    """
    import sys
    import types

    if "antenv.axon_hooks" in sys.modules:
        return
    from trn_agent_boot.trn_boot import _ntff_profile_via_ctypes

    hook = _ntff_profile_via_ctypes("/opt/axon/libaxon_pjrt.so")
    mod = types.ModuleType("antenv.axon_hooks")
    mod.get_axon_ntff_profile_hook = lambda: hook
    mod.set_axon_ntff_profile_hook = lambda h: None
    sys.modules["antenv.axon_hooks"] = mod
    import antenv

    antenv.axon_hooks = mod


def kernel(x, kernel, bias, _trace=False, _trace_kwargs=None):
    x = np.ascontiguousarray(x, dtype=np.float32)
    kernel = np.ascontiguousarray(kernel, dtype=np.float32)
    bias = np.ascontiguousarray(bias, dtype=np.float32)
    assert x.shape == (B, C, IN)

    if _trace:
        _install_ntff_shim()
    nc = _get_nc()
    # Block-diagonal weight stacks: wstack[p, j, q] holds cat 2j at
    # [0:64, j, 0:64] and cat 2j+1 at [64:128, j, 64:128].
    wstack = np.zeros((128, C // 2, 128), dtype=np.float32)
    wstack[0:IN, :, 0:OUT] = kernel[0, 0::2].transpose(1, 0, 2)
    wstack[IN:128, :, OUT:128] = kernel[0, 1::2].transpose(1, 0, 2)
    ident = np.eye(128, dtype=np.float32)
    in_maps = [
        {
            "x": x[i * B_SHARD:(i + 1) * B_SHARD],
            "wstack": wstack,
            "bias": bias,
            "ident": ident,
        }
        for i in range(N_CORES)
    ]
    res = run_bass_kernel_spmd(
        nc, in_maps, core_ids=list(range(N_CORES)),
        trace=_trace, **(_trace_kwargs or {})
    )
    out = np.concatenate([res.results[i]["out"] for i in range(N_CORES)], axis=0)
    if _trace:
        _NC_CACHE["last_results"] = res
    return out
